# revision 1
# baseline (speedup 1.0000x reference)
"""GraphSAGE (gnn_message_passing) forward pass on 8 Trainium2 NeuronCores.

Sharding strategy (hardcoded): row-shard the 10000 nodes across 8 cores
(1250 each, padded to 1280).  Each core receives its shard of the adjacency
matrix pre-transposed ([10240, 1280] bf16, columns = this core's nodes) so
the aggregation matmuls stream contiguous natural-layout slabs.  Node
features are kept feature-major ([feat_part, node_free]) on-chip so every
linear layer is a natural matmul.  Between GNN layers the updated node
features are AllGathered (bf16, node-major) so every core sees all nodes
for the next aggregation.  Small weights / LSTM params are replicated.
"""

import os
from contextlib import ExitStack

import numpy as np
import ml_dtypes

import concourse.bass as bass
import concourse.bacc as bacc
import concourse.mybir as mybir
import concourse.tile as tile
from concourse.bass_utils import run_bass_kernel_spmd
from concourse.masks import make_identity

F32 = mybir.dt.float32
BF16 = mybir.dt.bfloat16
AX = mybir.AxisListType
OP = mybir.AluOpType
AF = mybir.ActivationFunctionType

# ---- problem constants (hardcoded per spec) ----
N = 10000        # nodes
NC = 8           # cores
NPC = 1250       # original nodes per core
PC = 1280        # padded nodes per core
NP = NC * PC     # padded total nodes = 10240
KT = NP // 128   # 80 contraction tiles
IT = PC // 128   # 10 node tiles per core
NFEAT = 2000
FPAD = 2048
FT = FPAD // 128  # 16
NH = 128
NHE = 64
NFE = 256
D = NH + NHE     # 192
NOUT = 20
L = 2
BN_EPS = 1e-5

# matmul free-dim chunks over PC (PSUM bank = 512 fp32)
CHUNKS = [(0, 512), (512, 512), (1024, 256)]

LAST_RESULT = None  # test.py reads exec_time info from here

_CACHED_NC = None


def _bf(a):
    return np.asarray(a, dtype=ml_dtypes.bfloat16)


def _f32(a):
    return np.ascontiguousarray(a, dtype=np.float32)


# --------------------------------------------------------------------------
# device program
# --------------------------------------------------------------------------

def _build_program():
    nc = bacc.Bacc("TRN2", target_bir_lowering=False, debug=False, num_devices=NC)

    def inp(name, shape, dtype):
        return nc.declare_dram_parameter(name, list(shape), dtype, isOutput=False)

    # per-core tensors
    d_adjT = inp("adjT", [NP, PC], BF16)
    d_xT = inp("xT", [FPAD, PC], BF16)
    d_embT = inp("embT", [NFE, PC], F32)
    d_rsb = inp("rsb", [128, PC], F32)          # 1/rowsum broadcast to 128 parts
    # replicated weights
    d_w_inT = inp("w_inT", [FPAD, NH], BF16)
    d_sc_in = inp("sc_in", [NH, 1], F32)
    d_sh_in = inp("sh_in", [NH, 1], F32)        # with b_in folded
    d_sc_in_h = inp("sc_in_h", [NH, 1], F32)    # 0.5 * sc_in (for JK mean)
    d_sh_in2 = inp("sh_in2", [NH, 1], F32)      # without linear bias
    d_wgs_sT = inp("wgs_sT", [L, NH, NH], F32)
    d_wgs_nT = inp("wgs_nT", [L, NH, NH], F32)
    d_bgs = inp("bgs", [NH, L], F32)
    d_wihT = inp("wihT", [L, NH, 4 * NH], BF16)
    d_whhT = inp("whhT", [L, NH, 4 * NH], BF16)
    d_blstm = inp("blstm", [NH, 2 * 4], F32)    # (l, gate) on free dim
    d_w_embT = inp("w_embT", [NFE, NHE], F32)
    d_sc_emb = inp("sc_emb", [NHE, 1], F32)
    d_sh_emb = inp("sh_emb", [NHE, 1], F32)
    d_w_fcT = inp("w_fcT", [D, D], F32)
    d_sc_fc_a = inp("sc_fc_a", [128, 1], F32)
    d_sh_fc_a = inp("sh_fc_a", [128, 1], F32)
    d_sc_fc_b = inp("sc_fc_b", [64, 1], F32)
    d_sh_fc_b = inp("sh_fc_b", [64, 1], F32)
    d_w_outT = inp("w_outT", [D, NOUT], F32)
    d_bout = inp("bout", [128, NOUT], F32)      # b_out broadcast across parts
    d_out = nc.declare_dram_parameter("out", [PC, NOUT], F32, isOutput=True)

    # internal DRAM for collectives (each gather split in 2 halves so the
    # second half's transfer overlaps aggregation on the first half)
    HT = IT // 2   # 5 local k-tiles per half
    bounce = [[nc.dram_tensor(f"bounce{l}_{h}", [HT, 128, 128], BF16)
               for h in range(2)] for l in range(L)]
    hg = [[nc.dram_tensor(f"hg{l}_{h}", [NC * HT, 128, 128], BF16,
                          addr_space="Shared") for h in range(2)]
          for l in range(L)]
    groups = [list(range(NC))]

    with tile.TileContext(nc) as tc, ExitStack() as top:
        const = top.enter_context(tc.tile_pool(name="const", bufs=1))
        persist = top.enter_context(tc.tile_pool(name="persist", bufs=1))
        tmpf = top.enter_context(tc.tile_pool(name="tmpf", bufs=1))
        slabp = top.enter_context(tc.tile_pool(name="slab", bufs=4))
        hnatp = top.enter_context(tc.tile_pool(name="hnat", bufs=1))

        # ---- load constants ----
        w_in = const.tile([128, FPAD], BF16, tag="w_in")
        nc.sync.dma_start(w_in.rearrange("p (t j) -> p t j", t=FT),
                          d_w_inT.ap().rearrange("(t p) j -> p t j", p=128))
        wgs_s = [const.tile([128, NH], F32, tag=f"wgs_s{l}", name=f"wgs_s{l}")
                 for l in range(L)]
        wgs_n = [const.tile([128, NH], F32, tag=f"wgs_n{l}", name=f"wgs_n{l}")
                 for l in range(L)]
        for l in range(L):
            nc.sync.dma_start(wgs_s[l], d_wgs_sT[l])
            nc.sync.dma_start(wgs_n[l], d_wgs_nT[l])
        bgs = const.tile([128, L], F32, tag="bgs")
        nc.sync.dma_start(bgs, d_bgs.ap())
        wih = [const.tile([128, 4 * NH], BF16, tag=f"wih{l}", name=f"wih{l}")
               for l in range(L)]
        whh = [const.tile([128, 4 * NH], BF16, tag=f"whh{l}", name=f"whh{l}")
               for l in range(L)]
        for l in range(L):
            nc.sync.dma_start(wih[l], d_wihT[l])
            nc.sync.dma_start(whh[l], d_whhT[l])
        blstm = const.tile([128, 8], F32, tag="blstm")
        nc.sync.dma_start(blstm, d_blstm.ap())
        w_emb = [const.tile([128, NHE], F32, tag=f"w_emb{t}", name=f"w_emb{t}")
                 for t in range(2)]
        for t in range(2):
            nc.sync.dma_start(w_emb[t], d_w_embT[t * 128:(t + 1) * 128, :])
        wfc_aa = const.tile([128, 128], F32, tag="wfc_aa")
        wfc_ba = const.tile([64, 128], F32, tag="wfc_ba")
        wfc_ab = const.tile([128, 64], F32, tag="wfc_ab")
        wfc_bb = const.tile([64, 64], F32, tag="wfc_bb")
        nc.sync.dma_start(wfc_aa, d_w_fcT[:128, :128])
        nc.sync.dma_start(wfc_ba, d_w_fcT[128:, :128])
        nc.sync.dma_start(wfc_ab, d_w_fcT[:128, 128:])
        nc.sync.dma_start(wfc_bb, d_w_fcT[128:, 128:])
        w_out_a = const.tile([128, NOUT], F32, tag="w_out_a")
        w_out_b = const.tile([64, NOUT], F32, tag="w_out_b")
        nc.sync.dma_start(w_out_a, d_w_outT[:128, :])
        nc.sync.dma_start(w_out_b, d_w_outT[128:, :])
        bout = const.tile([128, NOUT], F32, tag="bout")
        nc.sync.dma_start(bout, d_bout.ap())
        rsb = const.tile([128, PC], F32, tag="rsb")
        nc.sync.dma_start(rsb, d_rsb.ap())

        small = {}
        for nm, dd, p in [
            ("sc_in", d_sc_in, NH), ("sh_in", d_sh_in, NH),
            ("sc_in_h", d_sc_in_h, NH), ("sh_in2", d_sh_in2, NH),
            ("sc_emb", d_sc_emb, NHE), ("sh_emb", d_sh_emb, NHE),
            ("sc_fc_a", d_sc_fc_a, 128), ("sh_fc_a", d_sh_fc_a, 128),
            ("sc_fc_b", d_sc_fc_b, 64), ("sh_fc_b", d_sh_fc_b, 64),
        ]:
            t = const.tile([p, 1], F32, tag=nm, name=nm)
            nc.sync.dma_start(t, dd.ap())
            small[nm] = t

        ident_bf = const.tile([128, 128], BF16, tag="ident")
        make_identity(nc, ident_bf)
        ones_col = const.tile([128, 1], F32, tag="ones_col")
        nc.vector.memset(ones_col, 1.0)
        ones_row = const.tile([1, 128], F32, tag="ones_row")
        nc.vector.memset(ones_row, 1.0)

        # ---- helpers ----
        def mm_acc(psum_ap, lhsT, rhs, start, stop):
            """accumulate lhsT.T @ rhs into psum, chunking free dim at 512"""
            F = rhs.shape[-1]
            o = 0
            while o < F:
                w = min(512, F - o)
                nc.tensor.matmul(
                    psum_ap[:, o:o + w], lhsT, rhs[:, o:o + w],
                    start=start, stop=stop,
                )
                o += w

        def elu_from(out_sb, in_ap, sc_ap, sh_ap):
            """out = elu(sc*in + sh); in_ap may be PSUM; [P, F]"""
            P, F = out_sb.shape[0], out_sb.shape[-1]
            y = tmpf.tile([128, PC], F32, tag="elu_y", name="elu_y")[:P, :F]
            nc.vector.tensor_scalar(y, in_ap, sc_ap, sh_ap, OP.mult, OP.add)
            e = tmpf.tile([128, PC], F32, tag="elu_e", name="elu_e")[:P, :F]
            nc.vector.tensor_scalar_min(e, y, 0.0)
            nc.scalar.activation(e, e, AF.Exp)
            # y <- max(y,0) - 1   (in place)
            nc.vector.tensor_scalar(y, y, 0.0, -1.0, OP.max, OP.add)
            nc.vector.tensor_tensor(out_sb, y, e, OP.add)

        # persistent activations
        hT = [persist.tile([128, PC], F32, tag="hT", bufs=2, name=f"hT{l}")
              for l in range(3)]
        hT_bf = [persist.tile([128, PC], BF16, tag=f"hTbf{l}", name=f"hTbf{l}")
                 for l in range(3)]

        with tc.tile_pool(name="psA", bufs=1, space="PSUM") as psA, \
             tc.tile_pool(name="psS", bufs=1, space="PSUM") as psS, \
             tc.tile_pool(name="psB", bufs=1, space="PSUM") as psB, \
             tc.tile_pool(name="psT", bufs=2, space="PSUM") as psT, \
             tc.tile_pool(name="tmpc", bufs=2) as tmpc:

            def gather(l, src_bf):
                """transpose local hT bf16 to node-major, AllGather (2 halves)"""
                for h in range(2):
                    loc = tmpc.tile([128, HT * 128], BF16, tag="hnat_loc",
                                    name="hnat_loc")
                    for s in range(HT):
                        it = h * HT + s
                        pt = psT.tile([128, 128], BF16, tag="tp", name="tp")
                        nc.tensor.transpose(
                            pt, src_bf[:, it * 128:(it + 1) * 128], ident_bf)
                        nc.vector.tensor_copy(loc[:, s * 128:(s + 1) * 128], pt)
                    nc.sync.dma_start(
                        bounce[l][h].ap().rearrange("t p f -> p t f"),
                        loc.rearrange("p (t f) -> p t f", t=HT))
                    nc.gpsimd.collective_compute(
                        "AllGather", OP.bypass, replica_groups=groups,
                        ins=[bounce[l][h].ap().opt()],
                        outs=[hg[l][h].ap().opt()],
                    )

            # ---- input projection: h0T = elu(bn(W_in @ x)) ----
            ps = psA.tile([128, PC], F32, tag="big", name="ps_proj")
            for t in range(FT):
                xt = tmpf.tile([128, PC], BF16, tag="xstream", bufs=3,
                               name="xt")
                nc.sync.dma_start(xt, d_xT[t * 128:(t + 1) * 128, :])
                mm_acc(ps, w_in[:, t * 128:(t + 1) * 128], xt,
                       start=(t == 0), stop=(t == FT - 1))
            elu_from(hT[0], ps, small["sc_in"], small["sh_in"])
            nc.vector.tensor_copy(hT_bf[0], hT[0])
            gather(0, hT_bf[0])

            # ---- GNN layers ----
            for l in range(L):
                ps_agg = psA.tile([128, PC], F32, tag="big", name="ps_agg")
                for h in range(2):
                    hnat = hnatp.tile([128, NC * HT * 128], BF16, tag="hnat",
                                      bufs=2, name="hnat")
                    nc.sync.dma_start(
                        hnat.rearrange("p (t f) -> p t f", t=NC * HT),
                        hg[l][h].ap().rearrange("t p f -> p t f"))
                    for r in range(NC):
                        row0 = r * PC + h * HT * 128
                        slab = slabp.tile([128, HT, PC], BF16, tag="slab",
                                          bufs=2, name="slab")
                        nc.sync.dma_start(
                            slab,
                            d_adjT[row0:row0 + HT * 128, :].rearrange(
                                "(s p) i -> p s i", p=128))
                        for s in range(HT):
                            t = r * HT + s
                            mm_acc(ps_agg, hnat[:, t * 128:(t + 1) * 128],
                                   slab[:, s, :],
                                   start=(h == 0 and r == 0 and s == 0),
                                   stop=(h == 1 and r == NC - 1 and s == HT - 1))
                neighT = tmpf.tile([128, PC], F32, tag="neighT", name="neighT")
                nc.vector.tensor_tensor(neighT, ps_agg, rsb, OP.mult)

                # GS linear: relu(W_self @ h + W_neigh @ neigh + b)
                ps_gs = psA.tile([128, PC], F32, tag="big", name="ps_gs")
                mm_acc(ps_gs, wgs_s[l], hT[l], start=True, stop=False)
                mm_acc(ps_gs, wgs_n[l], neighT, start=False, stop=True)
                hrelu = tmpf.tile([128, PC], F32, tag="hrelu", name="hrelu")
                nc.scalar.activation(hrelu, ps_gs, AF.Relu,
                                     bias=bgs[:, l:l + 1], scale=1.0)

                # L2 normalize along features (partition dim) via PE ones-reduce
                sq = tmpf.tile([128, PC], F32, tag="sq", name="sq")
                nc.vector.tensor_tensor(sq, hrelu, hrelu, OP.mult)
                nrm = tmpf.tile([1, PC], F32, tag="nrm", name="nrm")
                for (o, w) in CHUNKS:
                    ps_ss = psS.tile([1, 512], F32, tag="ss", name="ps_ss")
                    nc.tensor.matmul(ps_ss[:, :w], ones_col, sq[:, o:o + w],
                                     start=True, stop=True)
                    nc.scalar.activation(nrm[:, o:o + w], ps_ss[:, :w], AF.Sqrt)
                nc.vector.tensor_scalar_max(nrm, nrm, 1e-12)
                rec = tmpf.tile([1, PC], F32, tag="rec", name="rec")
                nc.vector.reciprocal(rec, nrm)
                for (o, w) in CHUNKS:
                    ps_bc = psB.tile([128, 512], F32, tag="bc", name="ps_bc")
                    nc.tensor.matmul(ps_bc[:, :w], ones_row, rec[:, o:o + w],
                                     start=True, stop=True)
                    nc.vector.tensor_tensor(hT[l + 1][:, o:o + w],
                                            hrelu[:, o:o + w], ps_bc[:, :w],
                                            OP.mult)
                nc.vector.tensor_copy(hT_bf[l + 1], hT[l + 1])
                if l == 0:
                    gather(1, hT_bf[1])

        # ---- 2-layer LSTM jumping knowledge over T=2 ----
        c_st = [persist.tile([128, PC], F32, tag=f"c{l}", name=f"c{l}")
                for l in range(2)]
        o_bf = [persist.tile([128, PC], BF16, tag=f"o{t}", name=f"o{t}")
                for t in range(2)]
        p0_bf = persist.tile([128, PC], BF16, tag="p0bf")
        p_f = [persist.tile([128, PC], F32, tag=f"p{t}f", name=f"p{t}f")
               for t in range(2)]

        with tc.tile_pool(name="psL", bufs=2, space="PSUM") as psL, \
             tc.tile_pool(name="tmpg", bufs=1) as tmpg:

            def lstm_cell(l, t, xin_bf, hprev_bf, c_tile, out_f32, out_bf):
                for (o, w) in CHUNKS:
                    gps = [psL.tile([128, 512], F32, tag=f"g{g}", name=f"g{g}")
                           for g in range(4)]
                    for g in range(4):
                        nc.tensor.matmul(
                            gps[g][:, :w],
                            wih[l][:, g * 128:(g + 1) * 128],
                            xin_bf[:, o:o + w],
                            start=True, stop=(t == 0))
                        if t > 0:
                            nc.tensor.matmul(
                                gps[g][:, :w],
                                whh[l][:, g * 128:(g + 1) * 128],
                                hprev_bf[:, o:o + w],
                                start=False, stop=True)
                    gact = []
                    for g, fn in enumerate([AF.Sigmoid, AF.Sigmoid,
                                            AF.Tanh, AF.Sigmoid]):
                        gt = tmpg.tile([128, 512], F32, tag=f"ga{g}",
                                       name=f"ga{g}")[:, :w]
                        nc.scalar.activation(gt, gps[g][:, :w], fn,
                                             bias=blstm[:, l * 4 + g:l * 4 + g + 1])
                        gact.append(gt)
                    ig, fg, gg, og = gact
                    cs = c_tile[:, o:o + w]
                    if t == 0:
                        nc.vector.tensor_tensor(cs, ig, gg, OP.mult)
                    else:
                        fc_ = tmpg.tile([128, 512], F32, tag="fc_",
                                        name="fc_")[:, :w]
                        nc.vector.tensor_tensor(fc_, fg, cs, OP.mult)
                        igg = tmpg.tile([128, 512], F32, tag="igg",
                                        name="igg")[:, :w]
                        nc.vector.tensor_tensor(igg, ig, gg, OP.mult)
                        nc.vector.tensor_tensor(cs, fc_, igg, OP.add)
                    tc_ = tmpg.tile([128, 512], F32, tag="tc_",
                                    name="tc_")[:, :w]
                    nc.scalar.activation(tc_, cs, AF.Tanh)
                    if out_f32 is not None:
                        nc.vector.tensor_tensor(out_f32[:, o:o + w], og, tc_,
                                                OP.mult)
                        if out_bf is not None:
                            nc.vector.tensor_copy(out_bf[:, o:o + w],
                                                  out_f32[:, o:o + w])
                    else:
                        nc.vector.tensor_tensor(out_bf[:, o:o + w], og, tc_,
                                                OP.mult)

            # layer0 t0; layer1 t0; layer0 t1; layer1 t1
            lstm_cell(0, 0, hT_bf[1], None, c_st[0], None, o_bf[0])
            lstm_cell(1, 0, o_bf[0], None, c_st[1], p_f[0], p0_bf)
            lstm_cell(0, 1, hT_bf[2], o_bf[0], c_st[0], None, o_bf[1])
            lstm_cell(1, 1, o_bf[1], p0_bf, c_st[1], p_f[1], None)

        # ---- post: JK mean -> bn/elu ; embed ; fc ; logits ; log_softmax ----
        hpost = persist.tile([128, PC], F32, tag="hpost")
        eT = persist.tile([64, PC], F32, tag="eT")
        hfca = persist.tile([128, PC], F32, tag="hfca")
        hfcb = persist.tile([64, PC], F32, tag="hfcb")
        outall = persist.tile([128, IT * NOUT], F32, tag="outall")

        with tc.tile_pool(name="psP", bufs=2, space="PSUM") as psP, \
             tc.tile_pool(name="psG", bufs=2, space="PSUM") as psG, \
             tc.tile_pool(name="tmps", bufs=2) as tmps:

            hsum = tmpf.tile([128, PC], F32, tag="neighT", name="hsum")
            nc.vector.tensor_tensor(hsum, p_f[0], p_f[1], OP.add)
            # 0.5 from the mean is folded into sc_in_h
            elu_from(hpost, hsum, small["sc_in_h"], small["sh_in2"])

            # embed projection
            ps_e = psP.tile([128, PC], F32, tag="post", name="ps_e")
            for t in range(2):
                et = tmpf.tile([128, PC], F32, tag="sq", name="et")
                nc.sync.dma_start(et, d_embT[t * 128:(t + 1) * 128, :])
                mm_acc(ps_e[:64, :], w_emb[t], et, start=(t == 0), stop=(t == 1))
            elu_from(eT, ps_e[:64, :], small["sc_emb"], small["sh_emb"])

            # fc on concat([hpost, eT]) without materializing the concat
            ps_fa = psP.tile([128, PC], F32, tag="post", name="ps_fa")
            mm_acc(ps_fa, wfc_aa, hpost, start=True, stop=False)
            mm_acc(ps_fa, wfc_ba, eT, start=False, stop=True)
            elu_from(hfca, ps_fa, small["sc_fc_a"], small["sh_fc_a"])
            ps_fb = psP.tile([128, PC], F32, tag="post", name="ps_fb")
            mm_acc(ps_fb[:64, :], wfc_ab, hpost, start=True, stop=False)
            mm_acc(ps_fb[:64, :], wfc_bb, eT, start=False, stop=True)
            elu_from(hfcb, ps_fb[:64, :], small["sc_fc_b"], small["sh_fc_b"])

            # logits per node-tile (natural orientation) + log_softmax
            for it in range(IT):
                ps_lg = psG.tile([128, NOUT], F32, tag="lg", name="ps_lg")
                nc.tensor.matmul(ps_lg, hfca[:, it * 128:(it + 1) * 128],
                                 w_out_a, start=True, stop=False)
                nc.tensor.matmul(ps_lg, hfcb[:, it * 128:(it + 1) * 128],
                                 w_out_b, start=False, stop=True)
                lg = tmps.tile([128, NOUT], F32, tag="lg_sb", name="lg_sb")
                nc.vector.tensor_tensor(lg, ps_lg, bout, OP.add)
                mx = tmps.tile([128, 1], F32, tag="mx", name="mx")
                nc.vector.tensor_reduce(mx, lg, AX.X, OP.max)
                sh = tmps.tile([128, NOUT], F32, tag="shift", name="shifted")
                nc.vector.tensor_scalar(sh, lg, mx, None, OP.subtract)
                ex = tmps.tile([128, NOUT], F32, tag="ex", name="ex")
                se = tmps.tile([128, 1], F32, tag="se", name="se")
                nc.scalar.activation(ex, sh, AF.Exp, accum_out=se)
                lse = tmps.tile([128, 1], F32, tag="lse", name="lse")
                nc.scalar.activation(lse, se, AF.Ln)
                nc.vector.tensor_scalar(
                    outall[:, it * NOUT:(it + 1) * NOUT], sh, lse, None,
                    OP.subtract)

            nc.sync.dma_start(
                d_out.ap().rearrange("(t p) c -> p t c", p=128),
                outall.rearrange("p (t c) -> p t c", t=IT))

    nc.compile()
    return nc


# --------------------------------------------------------------------------
# host side
# --------------------------------------------------------------------------

def _stage_inputs(
    x, embed, adj, W_in, b_in, bn_in_g, bn_in_b, bn_in_rm, bn_in_rv,
    W_gs, b_gs, Wih0, Whh0, bih0, bhh0, Wih1, Whh1, bih1, bhh1,
    W_emb, b_emb, bn_emb_g, bn_emb_b, bn_emb_rm, bn_emb_rv,
    W_fc, b_fc, bn_fc_g, bn_fc_b, bn_fc_rm, bn_fc_rv, W_out, b_out,
):
    x = np.asarray(x, np.float32)
    embed = np.asarray(embed, np.float32)
    adj = np.asarray(adj, np.float32)

    # replicated weight staging
    w_inT = np.zeros((FPAD, NH), ml_dtypes.bfloat16)
    w_inT[:NFEAT] = _bf(np.asarray(W_in, np.float32).T)

    def bn_fold(g, b, rm, rv, lin_b=None):
        g = np.asarray(g, np.float32); b = np.asarray(b, np.float32)
        rm = np.asarray(rm, np.float32); rv = np.asarray(rv, np.float32)
        sc = g / np.sqrt(rv + BN_EPS)
        base = lin_b if lin_b is not None else 0.0
        shv = sc * (base - rm) + b
        return _f32(sc), _f32(shv)

    sc_in, sh_in = bn_fold(bn_in_g, bn_in_b, bn_in_rm, bn_in_rv,
                           np.asarray(b_in, np.float32))
    _, sh_in2 = bn_fold(bn_in_g, bn_in_b, bn_in_rm, bn_in_rv)
    sc_emb, sh_emb = bn_fold(bn_emb_g, bn_emb_b, bn_emb_rm, bn_emb_rv,
                             np.asarray(b_emb, np.float32))
    sc_fc, sh_fc = bn_fold(bn_fc_g, bn_fc_b, bn_fc_rm, bn_fc_rv,
                           np.asarray(b_fc, np.float32))

    W_gs = np.asarray(W_gs, np.float32)
    wgs_sT = _f32(np.stack([W_gs[l][:, :NH].T for l in range(L)]))
    wgs_nT = _f32(np.stack([W_gs[l][:, NH:].T for l in range(L)]))
    bgs = _f32(np.asarray(b_gs, np.float32).T)          # [NH, L]

    wihT = np.stack([_bf(np.asarray(Wih0, np.float32).T),
                     _bf(np.asarray(Wih1, np.float32).T)])
    whhT = np.stack([_bf(np.asarray(Whh0, np.float32).T),
                     _bf(np.asarray(Whh1, np.float32).T)])
    bl = np.stack([np.asarray(bih0, np.float32) + np.asarray(bhh0, np.float32),
                   np.asarray(bih1, np.float32) + np.asarray(bhh1, np.float32)])
    # [512] per layer -> [128, l*4+g]
    blstm = np.zeros((NH, 8), np.float32)
    for l in range(2):
        for g in range(4):
            blstm[:, l * 4 + g] = bl[l][g * NH:(g + 1) * NH]

    w_embT = _f32(np.asarray(W_emb, np.float32).T)
    w_fcT = _f32(np.asarray(W_fc, np.float32).T)
    w_outT = _f32(np.asarray(W_out, np.float32).T)
    bout = _f32(np.tile(np.asarray(b_out, np.float32)[None, :], (128, 1)))

    shared = {
        "w_inT": w_inT,
        "sc_in": sc_in[:, None], "sh_in": sh_in[:, None],
        "sc_in_h": _f32(0.5 * sc_in)[:, None], "sh_in2": sh_in2[:, None],
        "wgs_sT": wgs_sT, "wgs_nT": wgs_nT, "bgs": bgs,
        "wihT": _bf(wihT), "whhT": _bf(whhT), "blstm": blstm,
        "w_embT": w_embT, "sc_emb": sc_emb[:, None], "sh_emb": sh_emb[:, None],
        "w_fcT": w_fcT,
        "sc_fc_a": _f32(sc_fc[:128])[:, None], "sh_fc_a": _f32(sh_fc[:128])[:, None],
        "sc_fc_b": _f32(sc_fc[128:])[:, None], "sh_fc_b": _f32(sh_fc[128:])[:, None],
        "w_outT": w_outT, "bout": bout,
    }

    # adjacency: per-core transposed bf16 shard with padded global ordering
    adj_bf = _bf(adj)
    rowsum = adj.sum(axis=1)                     # fp32, exact rows
    in_maps = []
    for c in range(NC):
        rows = slice(c * NPC, (c + 1) * NPC)
        adjT = np.zeros((NP, PC), ml_dtypes.bfloat16)
        blk = adj_bf[rows].T                     # [10000, 1250] view
        for ck in range(NC):
            adjT[ck * PC:ck * PC + NPC, :NPC] = blk[ck * NPC:(ck + 1) * NPC]
        xT = np.zeros((FPAD, PC), ml_dtypes.bfloat16)
        xT[:NFEAT, :NPC] = _bf(x[rows].T)
        embT = np.zeros((NFE, PC), np.float32)
        embT[:, :NPC] = embed[rows].T
        rec = np.zeros((PC,), np.float32)
        rec[:NPC] = 1.0 / rowsum[rows]
        rsb = np.ascontiguousarray(
            np.broadcast_to(rec[None, :], (128, PC)), dtype=np.float32)
        m = {"adjT": adjT, "xT": xT, "embT": embT, "rsb": rsb}
        m.update(shared)
        in_maps.append(m)
    return in_maps


def kernel(**inputs) -> np.ndarray:
    global _CACHED_NC, LAST_RESULT
    in_maps = _stage_inputs(**inputs)
    if _CACHED_NC is None:
        _CACHED_NC = _build_program()
    nc = _CACHED_NC
    trace = bool(int(os.environ.get("GSAGE_TRACE", "0")))
    res = run_bass_kernel_spmd(
        nc, in_maps, core_ids=list(range(NC)), trace=trace,
    )
    LAST_RESULT = res
    out = np.concatenate(
        [res.results[c]["out"][:NPC] for c in range(NC)], axis=0)
    return np.ascontiguousarray(out, np.float32)


if __name__ == "__main__":
    import reference
    inputs = reference.setup_inputs()
    out = kernel(**{k: np.asarray(v) for k, v in inputs.items()})
    print("out", out.shape, out.dtype)



# revision 2
# speedup vs baseline: 1.5405x; 1.5405x over previous
"""GraphSAGE (gnn_message_passing) forward pass on 8 Trainium2 NeuronCores.

Sharding (hardcoded): row-shard the 10000 nodes across 8 cores (1250 each,
padded to 1280).  The row-normalized adjacency shard is staged host-side as
fp8e4m3 ([10240, 1280] transposed, scaled by 4096 with the inverse scale
folded into W_neigh) and loaded into SBUF once -- both GNN layers aggregate
from the same resident/streamed copy.  Node features travel between layers
via fp8 AllGathers (two halves each, pipelined against the aggregation
matmuls).  Small weights / LSTM params are replicated; LSTM t0 cells and the
embed projection run under the aggregation matmuls.
"""

import os
from contextlib import ExitStack

import numpy as np
import ml_dtypes

import concourse.bass as bass
import concourse.bacc as bacc
import concourse.mybir as mybir
import concourse.tile as tile
from concourse.bass_utils import run_bass_kernel_spmd
from concourse.masks import make_identity

F32 = mybir.dt.float32
BF16 = mybir.dt.bfloat16
FP8 = mybir.dt.float8e4
AX = mybir.AxisListType
OP = mybir.AluOpType
AF = mybir.ActivationFunctionType

# ---- problem constants (hardcoded per spec) ----
N = 10000        # nodes
NC = 8           # cores
NPC = 1250       # original nodes per core
PC = 1280        # padded nodes per core
NP = NC * PC     # padded total nodes = 10240
KT = NP // 128   # 80 contraction tiles
IT = PC // 128   # 10 node tiles per core
HT = 5           # k-tiles per gather half per core
NFEAT = 2000
FPAD = 2048
FT = FPAD // 128  # 16
NH = 128
NHE = 64
NFE = 256
D = NH + NHE     # 192
NOUT = 20
L = 2
BN_EPS = 1e-5
ADJ_SCALE = 4096.0
NRES = 4         # adjacency chunks resident in SBUF across both layers
NE = 8           # x eighths
EW = PC // NE    # 160

CHUNKS = [(0, 512), (512, 512), (1024, 256)]

LAST_RESULT = None  # test.py reads exec_time info from here

_CACHED_NC = None


def _bf(a):
    return np.asarray(a, dtype=ml_dtypes.bfloat16)


def _f32(a):
    return np.ascontiguousarray(a, dtype=np.float32)


# --------------------------------------------------------------------------
# device program
# --------------------------------------------------------------------------

def _build_program():
    nc = bacc.Bacc("TRN2", target_bir_lowering=False, debug=False, num_devices=NC)

    def inp(name, shape, dtype):
        return nc.declare_dram_parameter(name, list(shape), dtype, isOutput=False)

    # per-core tensors
    d_adj = inp("adj8", [2, NC, 128, HT, PC], FP8)   # [half, chunk, p, s, i]
    d_x = inp("x8", [NE, 128, FT, EW], BF16)
    d_emb = inp("embT", [128, 2, PC], BF16)
    # replicated weights
    d_w_inT = inp("w_inT", [FPAD, NH], BF16)
    d_wgs_sT = inp("wgs_sT", [L, NH, NH], BF16)
    d_wgs_nT = inp("wgs_nT", [L, NH, NH], BF16)      # pre-scaled by 1/ADJ_SCALE
    d_bgs = inp("bgs", [NH, L], F32)
    d_wihT = inp("wihT", [L, NH, 4 * NH], BF16)
    d_whhT = inp("whhT", [L, NH, 4 * NH], BF16)
    d_blstm = inp("blstm", [NH, 2 * 4], F32)
    d_w_embT = inp("w_embT", [NFE, NHE], BF16)
    d_w_fcT = inp("w_fcT", [D, D], BF16)
    d_w_outT = inp("w_outT", [D, NOUT], BF16)
    d_bout = inp("bout_col", [NOUT, 1], F32)
    d_sm = {}
    for nm, p in [("sc_in", NH), ("sh_in", NH), ("sc_in_h", NH), ("sh_in2", NH),
                  ("sc_emb", NHE), ("sh_emb", NHE),
                  ("sc_fc_a", 128), ("sh_fc_a", 128),
                  ("sc_fc_b", 64), ("sh_fc_b", 64)]:
        d_sm[nm] = inp(nm, [p, 1], F32)
    d_out = nc.declare_dram_parameter("out", [128, IT * NOUT], F32, isOutput=True)

    # internal DRAM for collectives
    bounce = [[nc.dram_tensor(f"bounce{l}_{h}", [128, HT * 128], FP8)
               for h in range(2)] for l in range(L)]
    hg = [[nc.dram_tensor(f"hg{l}_{h}", [NC, 128, HT * 128], FP8,
                          addr_space="Shared") for h in range(2)]
          for l in range(L)]
    groups = [list(range(NC))]

    with tile.TileContext(nc) as tc, ExitStack() as top:
        const = top.enter_context(tc.tile_pool(name="const", bufs=1))
        persist = top.enter_context(tc.tile_pool(name="persist", bufs=1))
        padjr = top.enter_context(tc.tile_pool(name="adjr", bufs=1))
        padjs = top.enter_context(tc.tile_pool(name="adjs", bufs=3))
        px = top.enter_context(tc.tile_pool(name="px", bufs=2))
        pnat = top.enter_context(tc.tile_pool(name="pnat", bufs=2))
        ploc = top.enter_context(tc.tile_pool(name="ploc", bufs=2))
        ptmp = top.enter_context(tc.tile_pool(name="ptmp", bufs=1))
        pg = top.enter_context(tc.tile_pool(name="pg", bufs=1))

        # ---- constants ----
        w_in = const.tile([128, FPAD], BF16, tag="w_in")
        nc.sync.dma_start(w_in.rearrange("p (t j) -> p t j", t=FT),
                          d_w_inT.ap().rearrange("(t p) j -> p t j", p=128))
        wgs_s = [const.tile([128, NH], BF16, tag=f"wgs_s{l}", name=f"wgs_s{l}")
                 for l in range(L)]
        wgs_n = [const.tile([128, NH], BF16, tag=f"wgs_n{l}", name=f"wgs_n{l}")
                 for l in range(L)]
        for l in range(L):
            nc.sync.dma_start(wgs_s[l], d_wgs_sT[l])
            nc.sync.dma_start(wgs_n[l], d_wgs_nT[l])
        bgs = const.tile([128, L], F32, tag="bgs")
        nc.sync.dma_start(bgs, d_bgs.ap())
        wih = [const.tile([128, 4 * NH], BF16, tag=f"wih{l}", name=f"wih{l}")
               for l in range(L)]
        whh = [const.tile([128, 4 * NH], BF16, tag=f"whh{l}", name=f"whh{l}")
               for l in range(L)]
        for l in range(L):
            nc.sync.dma_start(wih[l], d_wihT[l])
            nc.sync.dma_start(whh[l], d_whhT[l])
        blstm = const.tile([128, 8], F32, tag="blstm")
        nc.sync.dma_start(blstm, d_blstm.ap())
        w_emb = [const.tile([128, NHE], BF16, tag=f"w_emb{t}", name=f"w_emb{t}")
                 for t in range(2)]
        for t in range(2):
            nc.sync.dma_start(w_emb[t], d_w_embT[t * 128:(t + 1) * 128, :])
        wfc_aa = const.tile([128, 128], BF16, tag="wfc_aa")
        wfc_ba = const.tile([64, 128], BF16, tag="wfc_ba")
        wfc_ab = const.tile([128, 64], BF16, tag="wfc_ab")
        wfc_bb = const.tile([64, 64], BF16, tag="wfc_bb")
        nc.sync.dma_start(wfc_aa, d_w_fcT[:128, :128])
        nc.sync.dma_start(wfc_ba, d_w_fcT[128:, :128])
        nc.sync.dma_start(wfc_ab, d_w_fcT[:128, 128:])
        nc.sync.dma_start(wfc_bb, d_w_fcT[128:, 128:])
        w_out_a = const.tile([128, NOUT], BF16, tag="w_out_a")
        w_out_b = const.tile([64, NOUT], BF16, tag="w_out_b")
        nc.sync.dma_start(w_out_a, d_w_outT[:128, :])
        nc.sync.dma_start(w_out_b, d_w_outT[128:, :])
        bout_col = const.tile([NOUT, 1], F32, tag="bout_col")
        nc.sync.dma_start(bout_col, d_bout.ap())
        small = {}
        for nm, p in [("sc_in", NH), ("sh_in", NH), ("sc_in_h", NH),
                      ("sh_in2", NH), ("sc_emb", NHE), ("sh_emb", NHE),
                      ("sc_fc_a", 128), ("sh_fc_a", 128),
                      ("sc_fc_b", 64), ("sh_fc_b", 64)]:
            t = const.tile([p, 1], F32, tag=nm, name=nm)
            nc.sync.dma_start(t, d_sm[nm].ap())
            small[nm] = t

        ident_bf = const.tile([128, 128], BF16, tag="ident")
        make_identity(nc, ident_bf)
        ident20 = const.tile([NOUT, NOUT], F32, tag="ident20")
        make_identity(nc, ident20)
        ones_col_bf = const.tile([128, 1], BF16, tag="ones_col")
        nc.vector.memset(ones_col_bf, 1.0)
        ones_row = const.tile([1, 128], F32, tag="ones_row")
        nc.vector.memset(ones_row, 1.0)
        ones20_bf = const.tile([NOUT, 1], BF16, tag="ones20")
        nc.vector.memset(ones20_bf, 1.0)
        ones_r20 = const.tile([1, NOUT], F32, tag="ones_r20")
        nc.vector.memset(ones_r20, 1.0)

        # ---- bulk input DMAs (sync/SP ring; all DRAM-ready, no stalls) ----
        xq = []
        for e in range(NE):
            t = px.tile([128, FT, EW], BF16, tag="xq", name=f"xq{e}")
            nc.sync.dma_start(t, d_x[e])
            xq.append(t)
        embin = const.tile([128, 2, PC], BF16, tag="embin")
        nc.sync.dma_start(embin, d_emb.ap())
        # adjacency: residents first (both halves), then layer-0 stream slabs
        adjres = [padjr.tile([128, 2 * HT, PC], FP8, tag=f"adjres{g}",
                             name=f"adjres{g}") for g in range(NRES)]
        for g in range(NRES):
            for h in range(2):
                nc.sync.dma_start(adjres[g][:, h * HT:(h + 1) * HT, :],
                                  d_adj[h, g])
        stream = {}
        for h in range(2):
            for g in range(NRES, NC):
                t = padjs.tile([128, HT, PC], FP8, tag="adjst",
                               name=f"st0_{h}_{g}")
                nc.sync.dma_start(t, d_adj[h, g])
                stream[(0, h, g)] = t

        # persistent activations (bf16)
        hbf = [persist.tile([128, PC], BF16, tag=f"hbf{l}", name=f"hbf{l}")
               for l in range(3)]
        c_st = [persist.tile([128, PC], F32, tag=f"c{l}", name=f"c{l}")
                for l in range(2)]
        o_bf = [persist.tile([128, PC], BF16, tag=f"o{t}", name=f"o{t}")
                for t in range(2)]
        p_bf = [persist.tile([128, PC], BF16, tag=f"p{t}", name=f"p{t}")
                for t in range(2)]
        eT = persist.tile([64, PC], BF16, tag="eT")
        hpost = persist.tile([128, PC], BF16, tag="hpost")
        hfca = persist.tile([128, PC], BF16, tag="hfca")
        hfcb = persist.tile([64, PC], BF16, tag="hfcb")
        out_sb = persist.tile([NOUT, PC], F32, tag="out_sb")
        outall = persist.tile([128, IT * NOUT], F32, tag="outall")

        # tmps
        ta = ptmp.tile([128, PC], F32, tag="ta")         # f32 scratch
        te = ptmp.tile([128, PC], BF16, tag="te")        # elu exp scratch
        tnb = ptmp.tile([128, PC], BF16, tag="tnb")      # neighbors bf16
        trl = ptmp.tile([128, PC], F32, tag="trl")       # gs relu out
        tsq = ptmp.tile([128, PC], BF16, tag="tsq")      # squared
        tnr = ptmp.tile([1, 512], F32, tag="tnr")        # norm
        trc = ptmp.tile([1, 512], F32, tag="trc")        # 1/norm
        ga = [pg.tile([128, 512], F32, tag=f"ga{g}", name=f"ga{g}")
              for g in range(4)]
        gtc = pg.tile([128, 512], F32, tag="gtc")
        gfc = pg.tile([128, 512], F32, tag="gfc")
        gig = pg.tile([128, 512], F32, tag="gig")
        ty = pg.tile([NOUT, 512], F32, tag="ty")
        tex = pg.tile([NOUT, 512], BF16, tag="tex")
        tlse = pg.tile([1, 512], F32, tag="tlse")

        hnat = {}

        def elu_chunk(dst, src_ap, sc, sh, tmp, P=128):
            """dst = elu(sc*src + sh); src may be PSUM; all [P, w]."""
            nc.vector.tensor_scalar(dst, src_ap, sc, sh, OP.mult, OP.add)
            nc.scalar.activation(tmp, src_ap, AF.Exp, bias=sh, scale=sc)
            nc.vector.tensor_scalar_min(tmp, tmp, 1.0)
            nc.vector.tensor_scalar(dst, dst, 0.0, -1.0, OP.max, OP.add)
            nc.vector.tensor_tensor(dst, dst, tmp, OP.add)

        with tc.tile_pool(name="psS", bufs=1, space="PSUM") as psS:

            def S(i):
                return psS.tile([128, 512], F32, tag=f"s{i}", name=f"s{i}")

            def gather(l, h, src_bf, tpool):
                """transpose h-half of src to node-major fp8, AllGather."""
                tp = tpool.tile([128, HT * 128], BF16, tag="tp", name="tp")
                for s in range(HT):
                    col = h * 640 + s * 128
                    nc.tensor.transpose(tp[:, s * 128:(s + 1) * 128],
                                        src_bf[:, col:col + 128], ident_bf)
                loc = ploc.tile([128, HT * 128], FP8, tag="loc", name="loc")
                nc.vector.tensor_copy(loc, tp)
                nc.scalar.dma_start(bounce[l][h].ap(), loc)
                nc.gpsimd.collective_compute(
                    "AllGather", OP.bypass, replica_groups=groups,
                    ins=[bounce[l][h].ap().opt()],
                    outs=[hg[l][h].ap().opt()],
                )
                hn = pnat.tile([128, NC, HT * 128], FP8, tag="hnat",
                               name=f"hnat{l}_{h}")
                nc.gpsimd.dma_start(
                    hn, hg[l][h].ap().rearrange("c p f -> p c f"))
                hnat[(l, h)] = hn

            def lstm_chunk(l, t, ci, xin, hprev, c_tile, out_tile):
                o, w = CHUNKS[ci]
                gates = (0, 2, 3) if t == 0 else (0, 1, 2, 3)
                gps = {}
                for g in gates:
                    ps = S(g)
                    nc.tensor.matmul(ps[:, :w],
                                     wih[l][:, g * 128:(g + 1) * 128],
                                     xin[:, o:o + w],
                                     start=True, stop=(t == 0))
                    if t:
                        nc.tensor.matmul(ps[:, :w],
                                         whh[l][:, g * 128:(g + 1) * 128],
                                         hprev[:, o:o + w],
                                         start=False, stop=True)
                    gps[g] = ps
                gact = {}
                for g in gates:
                    fn = AF.Tanh if g == 2 else AF.Sigmoid
                    gt = ga[g][:, :w]
                    nc.scalar.activation(gt, gps[g][:, :w], fn,
                                         bias=blstm[:, l * 4 + g:l * 4 + g + 1])
                    gact[g] = gt
                cs = c_tile[:, o:o + w]
                if t == 0:
                    nc.vector.tensor_tensor(cs, gact[0], gact[2], OP.mult)
                else:
                    nc.vector.tensor_tensor(gfc[:, :w], gact[1], cs, OP.mult)
                    nc.vector.tensor_tensor(gig[:, :w], gact[0], gact[2],
                                            OP.mult)
                    nc.vector.tensor_tensor(cs, gfc[:, :w], gig[:, :w], OP.add)
                nc.scalar.activation(gtc[:, :w], cs, AF.Tanh)
                nc.vector.tensor_tensor(out_tile[:, o:o + w], gact[3],
                                        gtc[:, :w], OP.mult)

            with tc.tile_pool(name="psB", bufs=1, space="PSUM") as psB, \
                 tc.tile_pool(name="psT", bufs=1, space="PSUM") as psT:

                # ---- input projection (eighths) + gather0 per half ----
                for half in range(2):
                    for e in range(half * 4, half * 4 + 4):
                        ps = S(e % 2)
                        for t in range(FT):
                            nc.tensor.matmul(
                                ps[:, :EW], w_in[:, t * 128:(t + 1) * 128],
                                xq[e][:, t, :],
                                start=(t == 0), stop=(t == FT - 1))
                        sl = slice(e * EW, (e + 1) * EW)
                        nc.vector.tensor_scalar(hbf[0][:, sl], ps[:, :EW],
                                                small["sc_in"], small["sh_in"],
                                                OP.mult, OP.add)
                        nc.scalar.activation(te[:, sl], ps[:, :EW], AF.Exp,
                                             bias=small["sh_in"],
                                             scale=small["sc_in"])
                    hs = slice(half * 640, half * 640 + 640)
                    nc.vector.tensor_scalar_min(te[:, hs], te[:, hs], 1.0)
                    nc.vector.tensor_scalar(hbf[0][:, hs], hbf[0][:, hs],
                                            0.0, -1.0, OP.max, OP.add)
                    nc.vector.tensor_tensor(hbf[0][:, hs], hbf[0][:, hs],
                                            te[:, hs], OP.add)
                    gather(0, half, hbf[0], psT)

                # ---- embed projection (during gather0 wait) ----
                for ci, (o, w) in enumerate(CHUNKS):
                    ps = S(2 + ci % 2)
                    nc.tensor.matmul(ps[:64, :w], w_emb[0],
                                     embin[:, 0, o:o + w], start=True,
                                     stop=False)
                    nc.tensor.matmul(ps[:64, :w], w_emb[1],
                                     embin[:, 1, o:o + w], start=False,
                                     stop=True)
                    elu_chunk(eT[:, o:o + w], ps[:64, :w], small["sc_emb"],
                              small["sh_emb"], te[:64, o:o + w], P=64)

                # ---- GNN layers ----
                for l in range(L):
                    # aggregation over all 80 k-tiles (both gather halves)
                    ps_agg = psB.tile([128, PC], F32, tag="agg", name="agg")
                    for h in range(2):
                        hn = hnat[(l, h)]
                        for g in range(NC):
                            if g < NRES:
                                rt = adjres[g]
                                rsl = lambda s, o, w: rt[:, h * HT + s, o:o + w]
                            else:
                                st = stream[(l, h, g)]
                                rsl = lambda s, o, w: st[:, s, o:o + w]
                            for s in range(HT):
                                lhsT = hn[:, g, s * 128:(s + 1) * 128]
                                for (o, w) in CHUNKS:
                                    nc.tensor.matmul(
                                        ps_agg[:, o:o + w], lhsT, rsl(s, o, w),
                                        start=(h == 0 and g == 0 and s == 0),
                                        stop=(h == 1 and g == NC - 1
                                              and s == HT - 1))
                    # stream next layer's non-resident slabs (layer 0 only)
                    if l == 0:
                        for h in range(2):
                            for g in range(NRES, NC):
                                t = padjs.tile([128, HT, PC], FP8, tag="adjst",
                                               name=f"st1_{h}_{g}")
                                nc.sync.dma_start(t, d_adj[h, g])
                                stream[(1, h, g)] = t

                    # gs linear + l2-normalize, per free chunk
                    for ci, (o, w) in enumerate(CHUNKS):
                        nc.vector.tensor_copy(tnb[:, o:o + w],
                                              ps_agg[:, o:o + w])
                        ps_gs = S(0)
                        nc.tensor.matmul(ps_gs[:, :w], wgs_s[l],
                                         hbf[l][:, o:o + w],
                                         start=True, stop=False)
                        nc.tensor.matmul(ps_gs[:, :w], wgs_n[l],
                                         tnb[:, o:o + w],
                                         start=False, stop=True)
                        nc.scalar.activation(trl[:, o:o + w], ps_gs[:, :w],
                                             AF.Relu, bias=bgs[:, l:l + 1])
                        nc.vector.tensor_tensor(tsq[:, o:o + w],
                                                trl[:, o:o + w],
                                                trl[:, o:o + w], OP.mult)
                        ps_ss = S(2)
                        nc.tensor.matmul(ps_ss[:1, :w], ones_col_bf,
                                         tsq[:, o:o + w], start=True,
                                         stop=True)
                        nc.scalar.activation(tnr[:, :w], ps_ss[:1, :w],
                                             AF.Sqrt)
                        nc.vector.tensor_scalar_max(tnr[:, :w], tnr[:, :w],
                                                    1e-12)
                        nc.vector.reciprocal(trc[:, :w], tnr[:, :w])
                        ps_bc = S(3)
                        nc.tensor.matmul(ps_bc[:, :w], ones_row, trc[:1, :w],
                                         start=True, stop=True)
                        nc.vector.tensor_tensor(hbf[l + 1][:, o:o + w],
                                                trl[:, o:o + w],
                                                ps_bc[:, :w], OP.mult)
                        # launch next gather as soon as a half is complete
                        if l == 0 and ci == 1:
                            gather(1, 0, hbf[1], psT)
                        if l == 0 and ci == 2:
                            gather(1, 1, hbf[1], psT)

                    # LSTM t=0 cells run under layer-1 aggregation
                    if l == 0:
                        for ci in range(3):
                            lstm_chunk(0, 0, ci, hbf[1], None, c_st[0], o_bf[0])
                        for ci in range(3):
                            lstm_chunk(1, 0, ci, o_bf[0], None, c_st[1], p_bf[0])

            # ---- tail: LSTM t=1, JK mean, embed concat, fc, log_softmax ----
            with tc.tile_pool(name="psO", bufs=1, space="PSUM") as psO:
                sm = psO.tile([1, 512], F32, tag="sm", name="sm")
                bc = psO.tile([NOUT, 512], F32, tag="bc", name="bc")
                tpo = psO.tile([128, IT * NOUT], F32, tag="tpo", name="tpo")

                for ci, (o, w) in enumerate(CHUNKS):
                    lstm_chunk(0, 1, ci, hbf[2], o_bf[0], c_st[0], o_bf[1])
                    lstm_chunk(1, 1, ci, o_bf[1], p_bf[0], c_st[1], p_bf[1])
                    # JK mean (0.5 folded into sc_in_h) -> bn -> elu
                    nc.vector.tensor_tensor(ta[:, o:o + w], p_bf[0][:, o:o + w],
                                            p_bf[1][:, o:o + w], OP.add)
                    elu_chunk(hpost[:, o:o + w], ta[:, o:o + w],
                              small["sc_in_h"], small["sh_in2"],
                              te[:, o:o + w])
                    # fc on concat([hpost, eT])
                    ps_fa = S(0)
                    nc.tensor.matmul(ps_fa[:, :w], wfc_aa, hpost[:, o:o + w],
                                     start=True, stop=False)
                    nc.tensor.matmul(ps_fa[:, :w], wfc_ba, eT[:, o:o + w],
                                     start=False, stop=True)
                    elu_chunk(hfca[:, o:o + w], ps_fa[:, :w],
                              small["sc_fc_a"], small["sh_fc_a"],
                              te[:, o:o + w])
                    ps_fb = S(1)
                    nc.tensor.matmul(ps_fb[:64, :w], wfc_ab, hpost[:, o:o + w],
                                     start=True, stop=False)
                    nc.tensor.matmul(ps_fb[:64, :w], wfc_bb, eT[:, o:o + w],
                                     start=False, stop=True)
                    elu_chunk(hfcb[:, o:o + w], ps_fb[:64, :w],
                              small["sc_fc_b"], small["sh_fc_b"],
                              te[:64, o:o + w], P=64)
                    # logits (feature-major) + log_softmax (no max-sub:
                    # |logits| < 3 measured, exp is safe in fp32)
                    ps_lg = S(2)
                    nc.tensor.matmul(ps_lg[:NOUT, :w], w_out_a,
                                     hfca[:, o:o + w], start=True, stop=False)
                    nc.tensor.matmul(ps_lg[:NOUT, :w], w_out_b,
                                     hfcb[:, o:o + w], start=False, stop=True)
                    nc.vector.tensor_scalar(ty[:, :w], ps_lg[:NOUT, :w],
                                            bout_col, None, OP.add)
                    nc.scalar.activation(tex[:, :w], ps_lg[:NOUT, :w], AF.Exp,
                                         bias=bout_col)
                    nc.tensor.matmul(sm[:1, :w], ones20_bf, tex[:, :w],
                                     start=True, stop=True)
                    nc.scalar.activation(tlse[:, :w], sm[:1, :w], AF.Ln)
                    nc.tensor.matmul(bc[:, :w], ones_r20, tlse[:1, :w],
                                     start=True, stop=True)
                    nc.vector.tensor_tensor(out_sb[:, o:o + w], ty[:, :w],
                                            bc[:, :w], OP.subtract)

                for it in range(IT):
                    nc.tensor.transpose(
                        tpo[:, it * NOUT:(it + 1) * NOUT],
                        out_sb[:, it * 128:(it + 1) * 128], ident20)
                nc.vector.tensor_copy(outall, tpo)
                nc.sync.dma_start(d_out.ap(), outall)

    nc.compile()
    return nc


# --------------------------------------------------------------------------
# host side
# --------------------------------------------------------------------------

def _stage_inputs(
    x, embed, adj, W_in, b_in, bn_in_g, bn_in_b, bn_in_rm, bn_in_rv,
    W_gs, b_gs, Wih0, Whh0, bih0, bhh0, Wih1, Whh1, bih1, bhh1,
    W_emb, b_emb, bn_emb_g, bn_emb_b, bn_emb_rm, bn_emb_rv,
    W_fc, b_fc, bn_fc_g, bn_fc_b, bn_fc_rm, bn_fc_rv, W_out, b_out,
):
    x = np.asarray(x, np.float32)
    embed = np.asarray(embed, np.float32)
    adj = np.asarray(adj, np.float32)

    w_inT = np.zeros((FPAD, NH), ml_dtypes.bfloat16)
    w_inT[:NFEAT] = _bf(np.asarray(W_in, np.float32).T)

    def bn_fold(g, b, rm, rv, lin_b=None):
        g = np.asarray(g, np.float32); b = np.asarray(b, np.float32)
        rm = np.asarray(rm, np.float32); rv = np.asarray(rv, np.float32)
        sc = g / np.sqrt(rv + BN_EPS)
        base = lin_b if lin_b is not None else 0.0
        shv = sc * (base - rm) + b
        return _f32(sc), _f32(shv)

    sc_in, sh_in = bn_fold(bn_in_g, bn_in_b, bn_in_rm, bn_in_rv,
                           np.asarray(b_in, np.float32))
    _, sh_in2 = bn_fold(bn_in_g, bn_in_b, bn_in_rm, bn_in_rv)
    sc_emb, sh_emb = bn_fold(bn_emb_g, bn_emb_b, bn_emb_rm, bn_emb_rv,
                             np.asarray(b_emb, np.float32))
    sc_fc, sh_fc = bn_fold(bn_fc_g, bn_fc_b, bn_fc_rm, bn_fc_rv,
                           np.asarray(b_fc, np.float32))

    W_gs = np.asarray(W_gs, np.float32)
    wgs_sT = _bf(np.stack([W_gs[l][:, :NH].T for l in range(L)]))
    wgs_nT = _bf(np.stack([W_gs[l][:, NH:].T for l in range(L)])
                 * (1.0 / ADJ_SCALE))
    bgs = _f32(np.asarray(b_gs, np.float32).T)          # [NH, L]

    wihT = np.stack([_bf(np.asarray(Wih0, np.float32).T),
                     _bf(np.asarray(Wih1, np.float32).T)])
    whhT = np.stack([_bf(np.asarray(Whh0, np.float32).T),
                     _bf(np.asarray(Whh1, np.float32).T)])
    bl = np.stack([np.asarray(bih0, np.float32) + np.asarray(bhh0, np.float32),
                   np.asarray(bih1, np.float32) + np.asarray(bhh1, np.float32)])
    blstm = np.zeros((NH, 8), np.float32)
    for l in range(2):
        for g in range(4):
            blstm[:, l * 4 + g] = bl[l][g * NH:(g + 1) * NH]

    shared = {
        "w_inT": w_inT,
        "wgs_sT": wgs_sT, "wgs_nT": wgs_nT, "bgs": bgs,
        "wihT": _bf(wihT), "whhT": _bf(whhT), "blstm": blstm,
        "w_embT": _bf(np.asarray(W_emb, np.float32).T),
        "w_fcT": _bf(np.asarray(W_fc, np.float32).T),
        "w_outT": _bf(np.asarray(W_out, np.float32).T),
        "bout_col": _f32(np.asarray(b_out, np.float32))[:, None],
        "sc_in": sc_in[:, None], "sh_in": sh_in[:, None],
        "sc_in_h": _f32(0.5 * sc_in)[:, None], "sh_in2": sh_in2[:, None],
        "sc_emb": sc_emb[:, None], "sh_emb": sh_emb[:, None],
        "sc_fc_a": _f32(sc_fc[:128])[:, None], "sh_fc_a": _f32(sh_fc[:128])[:, None],
        "sc_fc_b": _f32(sc_fc[128:])[:, None], "sh_fc_b": _f32(sh_fc[128:])[:, None],
    }

    rowsum = adj.sum(axis=1)                    # fp32, exact rows
    in_maps = []
    for c in range(NC):
        rows = slice(c * NPC, (c + 1) * NPC)
        scaled = adj[rows] * (ADJ_SCALE / rowsum[rows])[:, None]
        at = scaled.T                           # [10000, 1250]
        padded = np.zeros((NP, PC), np.float32)
        for ck in range(NC):
            padded[ck * PC:ck * PC + NPC, :NPC] = at[ck * NPC:(ck + 1) * NPC]
        adj8 = padded.astype(ml_dtypes.float8_e4m3fn)
        # [t, p, i] -> [h, g, p, s, i] with t = g*10 + h*5 + s
        adj8 = np.ascontiguousarray(
            adj8.reshape(NC, 2, HT, 128, PC).transpose(1, 0, 3, 2, 4))

        xT = np.zeros((FPAD, PC), ml_dtypes.bfloat16)
        xT[:NFEAT, :NPC] = _bf(x[rows].T)
        x8 = np.ascontiguousarray(
            xT.reshape(FT, 128, NE, EW).transpose(2, 1, 0, 3))

        embT = np.zeros((NFE, PC), ml_dtypes.bfloat16)
        embT[:, :NPC] = _bf(embed[rows].T)
        embT = np.ascontiguousarray(embT.reshape(2, 128, PC).transpose(1, 0, 2))

        m = {"adj8": adj8, "x8": x8, "embT": embT}
        m.update(shared)
        in_maps.append(m)
    return in_maps


def kernel(**inputs) -> np.ndarray:
    global _CACHED_NC, LAST_RESULT
    in_maps = _stage_inputs(**inputs)
    if _CACHED_NC is None:
        _CACHED_NC = _build_program()
    nc = _CACHED_NC
    trace = bool(int(os.environ.get("GSAGE_TRACE", "0")))
    res = run_bass_kernel_spmd(
        nc, in_maps, core_ids=list(range(NC)), trace=trace,
    )
    LAST_RESULT = res
    parts = []
    for c in range(NC):
        o = np.asarray(res.results[c]["out"], np.float32)
        o = o.reshape(128, IT, NOUT).transpose(1, 0, 2).reshape(PC, NOUT)
        parts.append(o[:NPC])
    out = np.concatenate(parts, axis=0)
    return np.ascontiguousarray(out, np.float32)


if __name__ == "__main__":
    import reference
    inputs = reference.setup_inputs()
    out = kernel(**{k: np.asarray(v) for k, v in inputs.items()})
    print("out", out.shape, out.dtype)


# revision 7
# speedup vs baseline: 1.6277x; 1.0566x over previous
"""GraphSAGE (gnn_message_passing) forward pass on 8 Trainium2 NeuronCores.

Sharding (hardcoded): row-shard the 10000 nodes across 8 cores (1250 each,
padded to 1280).  The row-normalized adjacency shard is staged host-side as
fp8e4m3 ([10240, 1280] transposed, scaled by 4096 with the inverse scale
folded into W_neigh) and loaded into SBUF once -- both GNN layers aggregate
from the same resident/streamed copy.  Node features travel between layers
via fp8 AllGathers (two halves each, pipelined against the aggregation
matmuls).  Small weights / LSTM params are replicated.

The LSTM is computed in "tanh-only" form (sigmoid(x) = 0.5*tanh(x/2)+0.5,
with the 0.5 factors folded into Whh/Wih1/biases and cell/h states kept
doubled) so the scalar engine never swaps activation tables inside the
recurrence; elu/softmax stages are likewise grouped by activation function
(activation-table loads cost ~1.3us each).
"""

import os
from contextlib import ExitStack

import numpy as np
import ml_dtypes

import concourse.bass as bass
import concourse.bacc as bacc
import concourse.mybir as mybir
import concourse.tile as tile
from concourse.bass_utils import run_bass_kernel_spmd
from concourse.masks import make_identity

F32 = mybir.dt.float32
BF16 = mybir.dt.bfloat16
FP8 = mybir.dt.float8e4
AX = mybir.AxisListType
OP = mybir.AluOpType
AF = mybir.ActivationFunctionType

# ---- problem constants (hardcoded per spec) ----
N = 10000        # nodes
NC = 8           # cores
NPC = 1250       # original nodes per core
PC = 1280        # padded nodes per core
NP = NC * PC     # padded total nodes = 10240
KT = NP // 128   # 80 contraction tiles
IT = PC // 128   # 10 node tiles per core
HT = 5           # k-tiles per gather half per core
NFEAT = 2000
FPAD = 2048
FT = FPAD // 128  # 16
NH = 128
NHE = 64
NFE = 256
D = NH + NHE     # 192
NOUT = 20
L = 2
BN_EPS = 1e-5
ADJ_SCALE = 4096.0
NRES = 4         # adjacency chunks resident in SBUF across both layers
NE = 8           # x eighths
EW = PC // NE    # 160

CHUNKS = [(0, 512), (512, 512), (1024, 256)]

LAST_RESULT = None  # test.py reads exec_time info from here

_CACHED_NC = None


def _bf(a):
    return np.asarray(a, dtype=ml_dtypes.bfloat16)


def _f32(a):
    return np.ascontiguousarray(a, dtype=np.float32)


# --------------------------------------------------------------------------
# device program
# --------------------------------------------------------------------------

def _build_program():
    nc = bacc.Bacc("TRN2", target_bir_lowering=False, debug=False, num_devices=NC)

    def inp(name, shape, dtype):
        return nc.declare_dram_parameter(name, list(shape), dtype, isOutput=False)

    # per-core tensors
    d_adj = inp("adj8", [2, NC, 128, HT, PC], FP8)   # [half, chunk, p, s, i]
    d_x = inp("x8", [NE, 128, FT, EW], BF16)
    d_emb = inp("embT", [128, 2, PC], BF16)
    # replicated weights
    d_w_inT = inp("w_inT", [FPAD, NH], BF16)
    d_wgs_sT = inp("wgs_sT", [L, NH, NH], BF16)
    d_wgs_nT = inp("wgs_nT", [L, NH, NH], BF16)      # pre-scaled by 1/ADJ_SCALE
    d_bgs = inp("bgs", [NH, L], F32)
    d_wihT = inp("wihT", [L, NH, 4 * NH], BF16)      # layer1 pre-scaled by 0.5
    d_whhT = inp("whhT", [L, NH, 4 * NH], BF16)      # pre-scaled by 0.5
    d_blstm = inp("blstm", [NH, 2 * 4], F32)         # i/f/o pre-scaled by 0.5
    d_w_embT = inp("w_embT", [NFE, NHE], BF16)
    d_w_fcT = inp("w_fcT", [D, D], BF16)
    d_w_outT = inp("w_outT", [D, NOUT], BF16)
    d_bout = inp("bout_col", [NOUT, 1], F32)
    d_sm = {}
    for nm, p in [("sc_in", NH), ("sh_in", NH), ("sc_in_h", NH), ("sh_in2", NH),
                  ("sc_emb", NHE), ("sh_emb", NHE),
                  ("sc_fc_a", 128), ("sh_fc_a", 128),
                  ("sc_fc_b", 64), ("sh_fc_b", 64)]:
        d_sm[nm] = inp(nm, [p, 1], F32)
    d_out = nc.declare_dram_parameter("out", [128, IT * NOUT], F32, isOutput=True)

    # internal DRAM for collectives
    bounce = [[nc.dram_tensor(f"bounce{l}_{h}", [128, HT * 128], FP8)
               for h in range(2)] for l in range(L)]
    hg = [[nc.dram_tensor(f"hg{l}_{h}", [NC, 128, HT * 128], FP8,
                          addr_space="Shared") for h in range(2)]
          for l in range(L)]
    groups = [list(range(NC))]

    with tile.TileContext(nc) as tc, ExitStack() as top:
        const = top.enter_context(tc.tile_pool(name="const", bufs=1))
        persist = top.enter_context(tc.tile_pool(name="persist", bufs=1))
        padjr = top.enter_context(tc.tile_pool(name="adjr", bufs=1))
        padjs = top.enter_context(tc.tile_pool(name="adjs", bufs=3))
        px = top.enter_context(tc.tile_pool(name="px", bufs=2))
        pnat = top.enter_context(tc.tile_pool(name="pnat", bufs=2))
        ploc = top.enter_context(tc.tile_pool(name="ploc", bufs=2))
        ptmp = top.enter_context(tc.tile_pool(name="ptmp", bufs=1))
        pg = top.enter_context(tc.tile_pool(name="pg", bufs=1))

        # ---- tiles for constants (DMAs issued in ring-order below) ----
        w_in = const.tile([128, FPAD], BF16, tag="w_in")
        wgs_s = [const.tile([128, NH], BF16, tag=f"wgs_s{l}", name=f"wgs_s{l}")
                 for l in range(L)]
        wgs_n = [const.tile([128, NH], BF16, tag=f"wgs_n{l}", name=f"wgs_n{l}")
                 for l in range(L)]
        bgs = const.tile([128, L], F32, tag="bgs")
        wih = [const.tile([128, 4 * NH], BF16, tag=f"wih{l}", name=f"wih{l}")
               for l in range(L)]
        whh = [const.tile([128, 4 * NH], BF16, tag=f"whh{l}", name=f"whh{l}")
               for l in range(L)]
        blstm = const.tile([128, 8], F32, tag="blstm")
        w_emb = [const.tile([128, NHE], BF16, tag=f"w_emb{t}", name=f"w_emb{t}")
                 for t in range(2)]
        wfc_aa = const.tile([128, 128], BF16, tag="wfc_aa")
        wfc_ba = const.tile([64, 128], BF16, tag="wfc_ba")
        wfc_ab = const.tile([128, 64], BF16, tag="wfc_ab")
        wfc_bb = const.tile([64, 64], BF16, tag="wfc_bb")
        w_out_a = const.tile([128, NOUT], BF16, tag="w_out_a")
        w_out_b = const.tile([64, NOUT], BF16, tag="w_out_b")
        bout_col = const.tile([NOUT, 1], F32, tag="bout_col")
        small = {}
        for nm, p in [("sc_in", NH), ("sh_in", NH), ("sc_in_h", NH),
                      ("sh_in2", NH), ("sc_emb", NHE), ("sh_emb", NHE),
                      ("sc_fc_a", 128), ("sh_fc_a", 128),
                      ("sc_fc_b", 64), ("sh_fc_b", 64)]:
            small[nm] = const.tile([p, 1], F32, tag=nm, name=nm)
        embin = const.tile([128, 2, PC], BF16, tag="embin")

        ident_bf = const.tile([128, 128], BF16, tag="ident")
        make_identity(nc, ident_bf)
        ident20 = const.tile([NOUT, NOUT], F32, tag="ident20")
        make_identity(nc, ident20)
        ones_col_bf = const.tile([128, 1], BF16, tag="ones_col")
        nc.vector.memset(ones_col_bf, 1.0)
        ones_row = const.tile([1, 128], F32, tag="ones_row")
        nc.vector.memset(ones_row, 1.0)
        ones20_bf = const.tile([NOUT, 1], BF16, tag="ones20")
        nc.vector.memset(ones20_bf, 1.0)
        ones_r20 = const.tile([1, NOUT], F32, tag="ones_r20")
        nc.vector.memset(ones_r20, 1.0)

        # ---- sync/SP ring, in consumption order: w_in, x, emb, consts, adj
        nc.sync.dma_start(w_in.rearrange("p (t j) -> p t j", t=FT),
                          d_w_inT.ap().rearrange("(t p) j -> p t j", p=128))
        xq = []
        for e in range(NE):
            t = px.tile([128, FT, EW], BF16, tag="xq", name=f"xq{e}")
            nc.sync.dma_start(t, d_x[e])
            xq.append(t)
        nc.sync.dma_start(embin, d_emb.ap())
        for l in range(L):
            nc.sync.dma_start(wgs_s[l], d_wgs_sT[l])
            nc.sync.dma_start(wgs_n[l], d_wgs_nT[l])
            nc.sync.dma_start(wih[l], d_wihT[l])
            nc.sync.dma_start(whh[l], d_whhT[l])
        nc.sync.dma_start(bgs, d_bgs.ap())
        nc.sync.dma_start(blstm, d_blstm.ap())
        for t in range(2):
            nc.sync.dma_start(w_emb[t], d_w_embT[t * 128:(t + 1) * 128, :])
        nc.sync.dma_start(wfc_aa, d_w_fcT[:128, :128])
        nc.sync.dma_start(wfc_ba, d_w_fcT[128:, :128])
        nc.sync.dma_start(wfc_ab, d_w_fcT[:128, 128:])
        nc.sync.dma_start(wfc_bb, d_w_fcT[128:, 128:])
        nc.sync.dma_start(w_out_a, d_w_outT[:128, :])
        nc.sync.dma_start(w_out_b, d_w_outT[128:, :])
        nc.sync.dma_start(bout_col, d_bout.ap())
        for nm in small:
            nc.sync.dma_start(small[nm], d_sm[nm].ap())
        # adjacency: residents first (both halves), then layer-0 stream slabs
        adjres = [padjr.tile([128, 2 * HT, PC], FP8, tag=f"adjres{g}",
                             name=f"adjres{g}") for g in range(NRES)]
        for g in range(NRES):
            for h in range(2):
                nc.sync.dma_start(adjres[g][:, h * HT:(h + 1) * HT, :],
                                  d_adj[h, g])
        stream = {}
        for h in range(2):
            for g in range(NRES, NC):
                t = padjs.tile([128, HT, PC], FP8, tag="adjst",
                               name=f"st0_{h}_{g}")
                nc.sync.dma_start(t, d_adj[h, g])
                stream[(0, h, g)] = t

        # persistent activations (bf16; LSTM h-states are kept DOUBLED)
        hbf = [persist.tile([128, PC], BF16, tag=f"hbf{l}", name=f"hbf{l}")
               for l in range(3)]
        c_st = [persist.tile([128, PC], F32, tag=f"c{l}", name=f"c{l}")
                for l in range(2)]
        o_bf = [persist.tile([128, PC], BF16, tag=f"o{t}", name=f"o{t}")
                for t in range(2)]
        p_bf = [persist.tile([128, PC], BF16, tag=f"p{t}", name=f"p{t}")
                for t in range(2)]
        eT = persist.tile([64, PC], BF16, tag="eT")
        hpost = persist.tile([128, PC], BF16, tag="hpost")
        hfca = persist.tile([128, PC], BF16, tag="hfca")
        hfcb = persist.tile([64, PC], BF16, tag="hfcb")
        out_sb = persist.tile([NOUT, PC], F32, tag="out_sb")
        outall = persist.tile([128, IT * NOUT], F32, tag="outall")

        # tmps
        tnb = ptmp.tile([128, PC], BF16, tag="tnb")      # neighbors / h-sum
        trl = ptmp.tile([128, PC], F32, tag="trl")       # gs relu out
        tsq = ptmp.tile([128, PC], BF16, tag="tsq")      # squared
        t3a = ptmp.tile([1, 3 * 512], F32, tag="t3a")    # norm / lse (chunks)
        t3b = ptmp.tile([1, 3 * 512], F32, tag="t3b")    # 1/norm
        ty3 = ptmp.tile([NOUT, 3 * 512], F32, tag="ty3")  # logits+bias
        ga = [pg.tile([128, 512], F32, tag=f"ga{g}", name=f"ga{g}")
              for g in range(4)]
        gtc = pg.tile([128, 512], F32, tag="gtc")
        gfc = pg.tile([128, 512], F32, tag="gfc")
        gig = pg.tile([128, 512], F32, tag="gig")
        pe1 = pg.tile([128, 512], F32, tag="pe1")   # post-pass elu scratch
        pe2 = pg.tile([128, 512], F32, tag="pe2")
        tex = pg.tile([NOUT, 512], BF16, tag="tex")

        hnat = {}

        def elu_chunk(dst, src_ap, sc, sh, ytmp, etmp):
            """dst = elu(sc*src + sh); src may be PSUM."""
            nc.vector.tensor_scalar(ytmp, src_ap, sc, sh, OP.mult, OP.add)
            nc.scalar.activation(etmp, src_ap, AF.Exp, bias=sh, scale=sc)
            nc.vector.tensor_scalar(etmp, etmp, 1.0, -1.0, OP.min, OP.add)
            nc.vector.scalar_tensor_tensor(dst, ytmp, 0.0, etmp, OP.max, OP.add)

        with tc.tile_pool(name="psS", bufs=1, space="PSUM") as psS:

            def S(i):
                return psS.tile([128, 512], F32, tag=f"s{i}", name=f"s{i}")

            def gather(l, h, src_bf, tpool):
                """transpose h-half of src to node-major fp8, AllGather."""
                tp = tpool.tile([128, HT * 128], BF16, tag="tp", name="tp")
                for s in range(HT):
                    col = h * 640 + s * 128
                    nc.tensor.transpose(tp[:, s * 128:(s + 1) * 128],
                                        src_bf[:, col:col + 128], ident_bf)
                loc = ploc.tile([128, HT * 128], FP8, tag="loc", name="loc")
                nc.vector.tensor_copy(loc, tp)
                nc.scalar.dma_start(bounce[l][h].ap(), loc)
                nc.gpsimd.collective_compute(
                    "AllGather", OP.bypass, replica_groups=groups,
                    ins=[bounce[l][h].ap().opt()],
                    outs=[hg[l][h].ap().opt()],
                )
                hn = pnat.tile([128, NC, HT * 128], FP8, tag="hnat",
                               name=f"hnat{l}_{h}")
                nc.gpsimd.dma_start(
                    hn, hg[l][h].ap().rearrange("c p f -> p c f"))
                hnat[(l, h)] = hn

            def lstm_chunk(l, t, ci, xin, hprev, c_tile, out_tile):
                """tanh-only LSTM cell chunk; c and h states are DOUBLED."""
                o, w = CHUNKS[ci]
                gates = (0, 2, 3) if t == 0 else (0, 1, 2, 3)
                gps = {}
                for g in gates:
                    ps = S(g)
                    nc.tensor.matmul(ps[:, :w],
                                     wih[l][:, g * 128:(g + 1) * 128],
                                     xin[:, o:o + w],
                                     start=True, stop=(t == 0))
                    if t:
                        nc.tensor.matmul(ps[:, :w],
                                         whh[l][:, g * 128:(g + 1) * 128],
                                         hprev[:, o:o + w],
                                         start=False, stop=True)
                    gps[g] = ps
                for g in gates:
                    # i/f/o: tanh(z/2 + b/2) = 2*sigmoid(z+b) - 1
                    nc.scalar.activation(ga[g][:, :w], gps[g][:, :w], AF.Tanh,
                                         bias=blstm[:, l * 4 + g:l * 4 + g + 1],
                                         scale=(1.0 if g == 2 else 0.5))
                cs = c_tile[:, o:o + w]
                if t == 0:
                    # c2 = (i~+1)*tanh(g)
                    nc.vector.scalar_tensor_tensor(cs, ga[0][:, :w], 1.0,
                                                   ga[2][:, :w], OP.add,
                                                   OP.mult)
                else:
                    nc.vector.scalar_tensor_tensor(gfc[:, :w], ga[1][:, :w],
                                                   1.0, cs, OP.add, OP.mult)
                    nc.vector.scalar_tensor_tensor(gig[:, :w], ga[0][:, :w],
                                                   1.0, ga[2][:, :w], OP.add,
                                                   OP.mult)
                    nc.vector.scalar_tensor_tensor(cs, gfc[:, :w], 0.5,
                                                   gig[:, :w], OP.mult, OP.add)
                nc.scalar.activation(gtc[:, :w], cs, AF.Tanh, scale=0.5)
                # h2 = (o~+1)*tanh(c)
                nc.vector.scalar_tensor_tensor(out_tile[:, o:o + w],
                                               ga[3][:, :w], 1.0, gtc[:, :w],
                                               OP.add, OP.mult)

            with tc.tile_pool(name="psB", bufs=1, space="PSUM") as psB, \
                 tc.tile_pool(name="psT", bufs=1, space="PSUM") as psT:

                # ---- input projection (eighths) + gather0 per half ----
                for half in range(2):
                    for q in range(2):
                        for j in range(2):
                            e = half * 4 + q * 2 + j
                            ps = S(j)
                            for t in range(FT):
                                nc.tensor.matmul(
                                    ps[:, :EW],
                                    w_in[:, t * 128:(t + 1) * 128],
                                    xq[e][:, t, :],
                                    start=(t == 0), stop=(t == FT - 1))
                            qs = slice(j * EW, (j + 1) * EW)
                            nc.vector.tensor_scalar(ga[0][:, qs], ps[:, :EW],
                                                    small["sc_in"],
                                                    small["sh_in"],
                                                    OP.mult, OP.add)
                            nc.scalar.activation(ga[1][:, qs], ps[:, :EW],
                                                 AF.Exp, bias=small["sh_in"],
                                                 scale=small["sc_in"])
                        qw = slice(0, 2 * EW)
                        dst = hbf[0][:, (half * 4 + q * 2) * EW:
                                     (half * 4 + q * 2 + 2) * EW]
                        nc.vector.tensor_scalar(ga[1][:, qw], ga[1][:, qw],
                                                1.0, -1.0, OP.min, OP.add)
                        nc.vector.scalar_tensor_tensor(dst, ga[0][:, qw], 0.0,
                                                       ga[1][:, qw], OP.max,
                                                       OP.add)
                    gather(0, half, hbf[0], psT)

                # ---- embed projection (during gather0 wait) ----
                for ci, (o, w) in enumerate(CHUNKS):
                    ps = S(2 + ci % 2)
                    nc.tensor.matmul(ps[:64, :w], w_emb[0],
                                     embin[:, 0, o:o + w], start=True,
                                     stop=False)
                    nc.tensor.matmul(ps[:64, :w], w_emb[1],
                                     embin[:, 1, o:o + w], start=False,
                                     stop=True)
                    elu_chunk(eT[:, o:o + w], ps[:64, :w], small["sc_emb"],
                              small["sh_emb"], gfc[:64, :w], gig[:64, :w])

                # ---- GNN layers ----
                for l in range(L):
                    # aggregation over all 80 k-tiles (both gather halves)
                    ps_agg = psB.tile([128, PC], F32, tag="agg", name="agg")
                    for h in range(2):
                        hn = hnat[(l, h)]
                        for g in range(NC):
                            for s in range(HT):
                                if g < NRES:
                                    rhs3 = adjres[g][:, h * HT + s, :]
                                else:
                                    rhs3 = stream[(l, h, g)][:, s, :]
                                lhsT = hn[:, g, s * 128:(s + 1) * 128]
                                for (o, w) in CHUNKS:
                                    nc.tensor.matmul(
                                        ps_agg[:, o:o + w], lhsT,
                                        rhs3[:, o:o + w],
                                        start=(h == 0 and g == 0 and s == 0),
                                        stop=(h == 1 and g == NC - 1
                                              and s == HT - 1))
                    # stream next layer's non-resident slabs (layer 0 only)
                    if l == 0:
                        for h in range(2):
                            for g in range(NRES, NC):
                                t = padjs.tile([128, HT, PC], FP8, tag="adjst",
                                               name=f"st1_{h}_{g}")
                                nc.sync.dma_start(t, d_adj[h, g])
                                stream[(1, h, g)] = t

                    # gs linear + relu + squared-sum (Relu table held)
                    sums = []
                    for ci, (o, w) in enumerate(CHUNKS):
                        nc.vector.tensor_copy(tnb[:, o:o + w],
                                              ps_agg[:, o:o + w])
                        ps_gs = S(0)
                        nc.tensor.matmul(ps_gs[:, :w], wgs_s[l],
                                         hbf[l][:, o:o + w],
                                         start=True, stop=False)
                        nc.tensor.matmul(ps_gs[:, :w], wgs_n[l],
                                         tnb[:, o:o + w],
                                         start=False, stop=True)
                        nc.scalar.activation(trl[:, o:o + w], ps_gs[:, :w],
                                             AF.Relu, bias=bgs[:, l:l + 1])
                        nc.vector.tensor_tensor(tsq[:, o:o + w],
                                                trl[:, o:o + w],
                                                trl[:, o:o + w], OP.mult)
                        ps_sum = S(1 + ci)
                        nc.tensor.matmul(ps_sum[:1, :w], ones_col_bf,
                                         tsq[:, o:o + w], start=True,
                                         stop=True)
                        sums.append(ps_sum)
                    # batched sqrt (one table load), then normalize
                    for ci, (o, w) in enumerate(CHUNKS):
                        nc.scalar.activation(t3a[:, ci * 512:ci * 512 + w],
                                             sums[ci][:1, :w], AF.Sqrt)
                    nc.vector.tensor_scalar_max(t3a, t3a, 1e-12)
                    nc.vector.reciprocal(t3b, t3a)
                    for ci, (o, w) in enumerate(CHUNKS):
                        ps_bc = S(1 + ci)
                        nc.tensor.matmul(ps_bc[:, :w], ones_row,
                                         t3b[:1, ci * 512:ci * 512 + w],
                                         start=True, stop=True)
                        nc.vector.tensor_tensor(hbf[l + 1][:, o:o + w],
                                                trl[:, o:o + w],
                                                ps_bc[:, :w], OP.mult)
                        if l == 0 and ci == 1:
                            gather(1, 0, hbf[1], psT)
                        if l == 0 and ci == 2:
                            gather(1, 1, hbf[1], psT)

                    # LSTM t=0 cells run under layer-1 aggregation
                    if l == 0:
                        for ci in range(3):
                            lstm_chunk(0, 0, ci, hbf[1], None, c_st[0], o_bf[0])
                        for ci in range(3):
                            lstm_chunk(1, 0, ci, o_bf[0], None, c_st[1], p_bf[0])

            # ---- tail: LSTM t=1 (all-Tanh pass), then Exp pass ----
            with tc.tile_pool(name="psO", bufs=1, space="PSUM") as psO:
                sm = [psO.tile([NOUT, 512], F32, tag=f"sm{c}", name=f"sm{c}")
                      for c in range(3)]
                tpo = psO.tile([128, IT * NOUT], F32, tag="tpo", name="tpo")

                for ci in range(3):
                    lstm_chunk(0, 1, ci, hbf[2], o_bf[0], c_st[0], o_bf[1])
                    lstm_chunk(1, 1, ci, o_bf[1], p_bf[0], c_st[1], p_bf[1])

                for ci, (o, w) in enumerate(CHUNKS):
                    # JK mean of doubled h's: 0.25 folded into sc_in_h
                    nc.vector.tensor_tensor(tnb[:, o:o + w], p_bf[0][:, o:o + w],
                                            p_bf[1][:, o:o + w], OP.add)
                    elu_chunk(hpost[:, o:o + w], tnb[:, o:o + w],
                              small["sc_in_h"], small["sh_in2"],
                              pe1[:, :w], pe2[:, :w])
                    # fc on concat([hpost, eT])
                    ps_fa = S(0)
                    nc.tensor.matmul(ps_fa[:, :w], wfc_aa, hpost[:, o:o + w],
                                     start=True, stop=False)
                    nc.tensor.matmul(ps_fa[:, :w], wfc_ba, eT[:, o:o + w],
                                     start=False, stop=True)
                    elu_chunk(hfca[:, o:o + w], ps_fa[:, :w],
                              small["sc_fc_a"], small["sh_fc_a"],
                              pe1[:, :w], pe2[:, :w])
                    ps_fb = S(1)
                    nc.tensor.matmul(ps_fb[:64, :w], wfc_ab, hpost[:, o:o + w],
                                     start=True, stop=False)
                    nc.tensor.matmul(ps_fb[:64, :w], wfc_bb, eT[:, o:o + w],
                                     start=False, stop=True)
                    elu_chunk(hfcb[:, o:o + w], ps_fb[:64, :w],
                              small["sc_fc_b"], small["sh_fc_b"],
                              pe1[:64, :w], pe2[:64, :w])
                    # logits (feature-major); |logits| < 3 so exp is safe
                    ps_lg = S(2)
                    nc.tensor.matmul(ps_lg[:NOUT, :w], w_out_a,
                                     hfca[:, o:o + w], start=True, stop=False)
                    nc.tensor.matmul(ps_lg[:NOUT, :w], w_out_b,
                                     hfcb[:, o:o + w], start=False, stop=True)
                    nc.vector.tensor_scalar(ty3[:, ci * 512:ci * 512 + w],
                                            ps_lg[:NOUT, :w],
                                            bout_col, None, OP.add)
                    nc.scalar.activation(tex[:, :w], ps_lg[:NOUT, :w], AF.Exp,
                                         bias=bout_col)
                    nc.tensor.matmul(sm[ci][:1, :w], ones20_bf, tex[:, :w],
                                     start=True, stop=True)
                # batched Ln (one table load), then bcast + subtract
                for ci, (o, w) in enumerate(CHUNKS):
                    nc.scalar.activation(t3a[:, ci * 512:ci * 512 + w],
                                         sm[ci][:1, :w], AF.Ln)
                for ci, (o, w) in enumerate(CHUNKS):
                    nc.tensor.matmul(sm[ci][:NOUT, :w], ones_r20,
                                     t3a[:1, ci * 512:ci * 512 + w],
                                     start=True, stop=True)
                    nc.vector.tensor_tensor(out_sb[:, o:o + w],
                                            ty3[:, ci * 512:ci * 512 + w],
                                            sm[ci][:NOUT, :w], OP.subtract)

                for it in range(IT):
                    nc.tensor.transpose(
                        tpo[:, it * NOUT:(it + 1) * NOUT],
                        out_sb[:, it * 128:(it + 1) * 128], ident20)
                nc.vector.tensor_copy(outall, tpo)
                nc.sync.dma_start(d_out.ap(), outall)

    nc.compile()
    return nc


# --------------------------------------------------------------------------
# host side
# --------------------------------------------------------------------------

def _stage_inputs(
    x, embed, adj, W_in, b_in, bn_in_g, bn_in_b, bn_in_rm, bn_in_rv,
    W_gs, b_gs, Wih0, Whh0, bih0, bhh0, Wih1, Whh1, bih1, bhh1,
    W_emb, b_emb, bn_emb_g, bn_emb_b, bn_emb_rm, bn_emb_rv,
    W_fc, b_fc, bn_fc_g, bn_fc_b, bn_fc_rm, bn_fc_rv, W_out, b_out,
):
    x = np.asarray(x, np.float32)
    embed = np.asarray(embed, np.float32)
    adj = np.asarray(adj, np.float32)

    w_inT = np.zeros((FPAD, NH), ml_dtypes.bfloat16)
    w_inT[:NFEAT] = _bf(np.asarray(W_in, np.float32).T)

    def bn_fold(g, b, rm, rv, lin_b=None):
        g = np.asarray(g, np.float32); b = np.asarray(b, np.float32)
        rm = np.asarray(rm, np.float32); rv = np.asarray(rv, np.float32)
        sc = g / np.sqrt(rv + BN_EPS)
        base = lin_b if lin_b is not None else 0.0
        shv = sc * (base - rm) + b
        return _f32(sc), _f32(shv)

    sc_in, sh_in = bn_fold(bn_in_g, bn_in_b, bn_in_rm, bn_in_rv,
                           np.asarray(b_in, np.float32))
    _, sh_in2 = bn_fold(bn_in_g, bn_in_b, bn_in_rm, bn_in_rv)
    sc_emb, sh_emb = bn_fold(bn_emb_g, bn_emb_b, bn_emb_rm, bn_emb_rv,
                             np.asarray(b_emb, np.float32))
    sc_fc, sh_fc = bn_fold(bn_fc_g, bn_fc_b, bn_fc_rm, bn_fc_rv,
                           np.asarray(b_fc, np.float32))

    W_gs = np.asarray(W_gs, np.float32)
    wgs_sT = _bf(np.stack([W_gs[l][:, :NH].T for l in range(L)]))
    wgs_nT = _bf(np.stack([W_gs[l][:, NH:].T for l in range(L)])
                 * (1.0 / ADJ_SCALE))
    bgs = _f32(np.asarray(b_gs, np.float32).T)          # [NH, L]

    # tanh-only LSTM: layer-1 inputs and all h_prev are DOUBLED h states,
    # so Wih1 and both Whh get 0.5 folded in; i/f/o biases are halved.
    wihT = np.stack([_bf(np.asarray(Wih0, np.float32).T),
                     _bf(0.5 * np.asarray(Wih1, np.float32).T)])
    whhT = np.stack([_bf(0.5 * np.asarray(Whh0, np.float32).T),
                     _bf(0.5 * np.asarray(Whh1, np.float32).T)])
    bl = np.stack([np.asarray(bih0, np.float32) + np.asarray(bhh0, np.float32),
                   np.asarray(bih1, np.float32) + np.asarray(bhh1, np.float32)])
    blstm = np.zeros((NH, 8), np.float32)
    for l in range(2):
        for g in range(4):
            f = 1.0 if g == 2 else 0.5
            blstm[:, l * 4 + g] = f * bl[l][g * NH:(g + 1) * NH]

    shared = {
        "w_inT": w_inT,
        "wgs_sT": wgs_sT, "wgs_nT": wgs_nT, "bgs": bgs,
        "wihT": _bf(wihT), "whhT": _bf(whhT), "blstm": blstm,
        "w_embT": _bf(np.asarray(W_emb, np.float32).T),
        "w_fcT": _bf(np.asarray(W_fc, np.float32).T),
        "w_outT": _bf(np.asarray(W_out, np.float32).T),
        "bout_col": _f32(np.asarray(b_out, np.float32))[:, None],
        "sc_in": sc_in[:, None], "sh_in": sh_in[:, None],
        # JK mean of two DOUBLED h states: 0.5 * 0.5 = 0.25
        "sc_in_h": _f32(0.25 * sc_in)[:, None], "sh_in2": sh_in2[:, None],
        "sc_emb": sc_emb[:, None], "sh_emb": sh_emb[:, None],
        "sc_fc_a": _f32(sc_fc[:128])[:, None], "sh_fc_a": _f32(sh_fc[:128])[:, None],
        "sc_fc_b": _f32(sc_fc[128:])[:, None], "sh_fc_b": _f32(sh_fc[128:])[:, None],
    }

    rowsum = adj.sum(axis=1)                    # fp32, exact rows
    in_maps = []
    for c in range(NC):
        rows = slice(c * NPC, (c + 1) * NPC)
        scaled = adj[rows] * (ADJ_SCALE / rowsum[rows])[:, None]
        at = scaled.T                           # [10000, 1250]
        padded = np.zeros((NP, PC), np.float32)
        for ck in range(NC):
            padded[ck * PC:ck * PC + NPC, :NPC] = at[ck * NPC:(ck + 1) * NPC]
        adj8 = padded.astype(ml_dtypes.float8_e4m3fn)
        # [t, p, i] -> [h, g, p, s, i] with t = g*10 + h*5 + s
        adj8 = np.ascontiguousarray(
            adj8.reshape(NC, 2, HT, 128, PC).transpose(1, 0, 3, 2, 4))

        xT = np.zeros((FPAD, PC), ml_dtypes.bfloat16)
        xT[:NFEAT, :NPC] = _bf(x[rows].T)
        x8 = np.ascontiguousarray(
            xT.reshape(FT, 128, NE, EW).transpose(2, 1, 0, 3))

        embT = np.zeros((NFE, PC), ml_dtypes.bfloat16)
        embT[:, :NPC] = _bf(embed[rows].T)
        embT = np.ascontiguousarray(embT.reshape(2, 128, PC).transpose(1, 0, 2))

        m = {"adj8": adj8, "x8": x8, "embT": embT}
        m.update(shared)
        in_maps.append(m)
    return in_maps


def kernel(**inputs) -> np.ndarray:
    global _CACHED_NC, LAST_RESULT
    in_maps = _stage_inputs(**inputs)
    if _CACHED_NC is None:
        _CACHED_NC = _build_program()
    nc = _CACHED_NC
    trace = bool(int(os.environ.get("GSAGE_TRACE", "0")))
    res = run_bass_kernel_spmd(
        nc, in_maps, core_ids=list(range(NC)), trace=trace,
    )
    LAST_RESULT = res
    parts = []
    for c in range(NC):
        o = np.asarray(res.results[c]["out"], np.float32)
        o = o.reshape(128, IT, NOUT).transpose(1, 0, 2).reshape(PC, NOUT)
        parts.append(o[:NPC])
    out = np.concatenate(parts, axis=0)
    return np.ascontiguousarray(out, np.float32)


if __name__ == "__main__":
    import reference
    inputs = reference.setup_inputs()
    out = kernel(**{k: np.asarray(v) for k, v in inputs.items()})
    print("out", out.shape, out.dtype)
